# revision 12
# baseline (speedup 1.0000x reference)
"""ConvVAE2d (patchify -> CVAE MLP encode/decode -> fold) on 8 TRN2 NeuronCores.

Data-parallel over batch: 1024 samples -> 8 cores x 128 samples.

Per-core dataflow (128 samples, 16 patches each -> 2048 MLP rows):
  All activations are kept transposed [feature, row] in SBUF, with rows
  ordered (ij, b) = patch-position-major. Weights act as the stationary
  matmul operand (lhsT), so every layer consumes and produces the same
  layout and no transposes are needed between layers.

  Phase A (per row-block i): DMA x i-slice -> PE-transpose 128x128 blocks
           into XT [768, 512] (patchify == transpose).
  Phase B: L1 relu(XT@W1 + crep@W1c + b1) -> H [1024, 512]
           L2 H@W2 + b2 -> ML [256, 512]   (mu = first 128 rows)
           L3 relu(mu@W3 + crep@W3c + b3) -> HD [1024, 512]
           L4 sigmoid(HD@W4 + b4) -> P [768, 512]
  Phase C: PE-transpose P / ML back to row-major staging, DMA out.

  Matmuls run as float32r (full PE speed, ~1e-4 rel err). Biases and
  activation functions are fused into the PSUM->SBUF eviction on ScalarE.
"""
import sys

try:
    import concourse.bass as bass
except ImportError:
    sys.path.insert(0, "/opt/trn_rl_repo")
    import concourse.bass as bass

import ml_dtypes
import numpy as np
import concourse.bacc as bacc
import concourse.mybir as mybir
import concourse.tile as tile
from concourse import bass_utils
from concourse.masks import make_identity

# Problem config (hardcoded; must match the reference)
BS, CH, IMG, K, NCLS, HID, LAT = 1024, 3, 64, 16, 10, 1024, 128
OH = IMG // K          # 4 patches per side
L = OH * OH            # 16 patches per image
PD = CH * K * K        # 768 flattened patch dim
N_CORES = 8
B = BS // N_CORES      # 128 samples per core
ROWS = B * L           # 2048 MLP rows per core
RB = 512               # rows per row-block (= 4 patch positions x 128 samples)

F32 = mybir.dt.float32
F32R = mybir.dt.float32r
BF16 = mybir.dt.bfloat16
MMDT = BF16  # matmul operand dtype (1 cyc/elem on PE; f32r runs at 2 cyc/elem)
AF = mybir.ActivationFunctionType

_cached = None


def _build():
    nc = bacc.Bacc("TRN2", target_bir_lowering=False, debug=False,
                   num_devices=N_CORES)

    def din(name, shape, dt=MMDT):
        return nc.dram_tensor(name, shape, dt, kind="ExternalInput").ap()

    x_d = din("x", [B, CH, IMG, IMG], F32)
    crep_d = din("crep", [NCLS, ROWS])
    w1_d = din("w1", [PD, HID])
    w1c_d = din("w1c", [NCLS, HID])
    w2_d = din("w2", [HID, 2 * LAT])
    w3_d = din("w3", [LAT, HID])
    w3c_d = din("w3c", [NCLS, HID])
    w4_d = din("w4", [HID, PD])
    b1_d = din("b1t", [128, HID // 128], F32)
    b2_d = din("b2t", [128, 2 * LAT // 128], F32)
    b3_d = din("b3t", [128, HID // 128], F32)
    b4_d = din("b4t", [128, PD // 128], F32)
    recon_d = nc.dram_tensor("recon", [B, CH, IMG, IMG], F32,
                             kind="ExternalOutput").ap()
    ml_d = nc.dram_tensor("ml", [B, 2 * LAT, OH, OH], F32,
                          kind="ExternalOutput").ap()

    NK1 = PD // 128    # 6 k-chunks for layer 1
    NN1 = HID // 128   # 8 n-chunks for layers 1/3
    NK2 = HID // 128   # 8
    NN2 = 2 * LAT // 128  # 2
    NN4 = PD // 128    # 6

    with tile.TileContext(nc) as tc:
        with (
            tc.tile_pool(name="const", bufs=1) as cp,
            tc.tile_pool(name="xnat", bufs=2) as xnp,
            tc.tile_pool(name="ximg", bufs=2) as xip,
            tc.tile_pool(name="xt", bufs=NK1 + 3) as xtp,
            tc.tile_pool(name="h", bufs=NN1 + 2) as hp,
            tc.tile_pool(name="ml", bufs=2 * NN2 + 1) as mlp,
            tc.tile_pool(name="hd", bufs=NN1 + 2) as hdp,
            tc.tile_pool(name="p", bufs=NN4 + 2) as pp,
            tc.tile_pool(name="prm", bufs=2) as prmp,
            tc.tile_pool(name="psmm", bufs=4, space="PSUM") as psmm,
            tc.tile_pool(name="pstp", bufs=4, space="PSUM") as pstp,
        ):
            # ---- constants ----
            # DMA issue order = first-use order: x row-block 0 first so the
            # patchify transposes start immediately, then per-layer weights
            # just-in-time (w1 before w2 before w3/w4).
            ident = cp.tile([128, 128], F32)
            make_identity(nc, ident[:, :])
            identm = cp.tile([128, 128], MMDT)
            make_identity(nc, identm[:, :])
            xnat0 = xnp.tile([128, CH, K, IMG], F32, tag="xnat", name="xnat0")
            w1 = cp.tile([128, NK1, HID], MMDT)
            w1v = w1_d.rearrange("(k p) n -> p k n", p=128)
            for ch in range(CH):
                nc.sync.dma_start(xnat0[:, ch, :, :], x_d[:, ch, 0:K, :])
                nc.sync.dma_start(w1[:, 2 * ch, :], w1v[:, 2 * ch, :])
                nc.sync.dma_start(w1[:, 2 * ch + 1, :], w1v[:, 2 * ch + 1, :])
            w1c = cp.tile([NCLS, HID], MMDT)
            nc.sync.dma_start(w1c[:, :], w1c_d)
            crep = cp.tile([NCLS, ROWS], MMDT)
            nc.sync.dma_start(crep[:, :], crep_d)
            b1t = cp.tile([128, NN1], F32)
            nc.sync.dma_start(b1t[:, :], b1_d)
            w2 = cp.tile([128, NK2, 2 * LAT], MMDT)
            nc.sync.dma_start(w2[:, :, :], w2_d.rearrange("(k p) n -> p k n", p=128))
            b2t = cp.tile([128, NN2], F32)
            nc.sync.dma_start(b2t[:, :], b2_d)
            w3 = cp.tile([128, HID], MMDT)
            nc.sync.dma_start(w3[:, :], w3_d)
            w3c = cp.tile([NCLS, HID], MMDT)
            nc.sync.dma_start(w3c[:, :], w3c_d)
            b3t = cp.tile([128, NN1], F32)
            nc.sync.dma_start(b3t[:, :], b3_d)
            w4 = cp.tile([128, NK2, PD], MMDT)
            nc.sync.dma_start(w4[:, :, :], w4_d.rearrange("(k p) n -> p k n", p=128))
            b4t = cp.tile([128, NN4], F32)
            nc.sync.dma_start(b4t[:, :], b4_d)
            # mu_logvar row-major staging, viewed [b, n, ij]
            mlrm = cp.tile([128, 2 * LAT, L], F32)

            for i in range(OH):  # row-block = one patch-row index i
                # -- Phase A: load + patchify-transpose --
                # One fully-contiguous DMA per row-block, then VectorE repacks
                # into (ch, j, rhalf, r8*16+col) so each transpose input is a
                # contiguous 128-elem slice. Doing the j-vs-(r,col) reorder in
                # DMA access patterns instead shreds it into 64B descriptors.
                if i == 0:
                    xnat = xnat0
                else:
                    xnat = xnp.tile([128, CH, K, IMG], F32, tag="xnat")
                    nc.sync.dma_start(xnat[:, :, :, :],
                                      x_d[:, :, K * i:K * (i + 1), :])
                ximg = xip.tile([128, CH, OH, 2, 128], MMDT, tag="ximg")
                for ch in range(CH):
                    for rh in range(2):
                        nc.vector.tensor_copy(
                            ximg[:, ch, :, rh, :].rearrange(
                                "b j (r c) -> b j r c", c=K),
                            xnat[:, ch, 8 * rh:8 * rh + 8, :].rearrange(
                                "b r (j c) -> b j r c", c=K))
                xt = [xtp.tile([128, RB], MMDT, tag="xt", name=f"xt{_k}") for _k in range(NK1)]
                for kc in range(NK1):
                    ch, rh = kc // 2, kc % 2
                    for j in range(OH):
                        ps = pstp.tile([128, 128], MMDT, tag="tp")
                        nc.tensor.transpose(
                            ps[:, :], ximg[:, ch, j, rh, :], identm[:, :])
                        nc.vector.tensor_copy(xt[kc][:, j * 128:(j + 1) * 128], ps[:, :])

                rbs = slice(i * RB, (i + 1) * RB)

                # -- L1: H = relu(XT@W1 + crep@W1c + b1) --
                h = [hp.tile([128, RB], MMDT, tag="h", name=f"h{_k}") for _k in range(NN1)]
                for n in range(NN1):
                    ps = psmm.tile([128, RB], F32, tag="mm")
                    for k in range(NK1):
                        nc.tensor.matmul(ps[:, :], w1[:, k, n * 128:(n + 1) * 128],
                                         xt[k][:, :], start=(k == 0), stop=False)
                    nc.tensor.matmul(ps[:, :], w1c[:, n * 128:(n + 1) * 128],
                                     crep[:, rbs], start=False, stop=True)
                    nc.scalar.activation(h[n][:, :], ps[:, :], AF.Relu,
                                         bias=b1t[:, n:n + 1])

                # -- L2: ML = H@W2 + b2 --
                ml = [mlp.tile([128, RB], F32, tag="ml", name=f"ml{_k}") for _k in range(NN2)]
                for n in range(NN2):
                    ps = psmm.tile([128, RB], F32, tag="mm")
                    for k in range(NK2):
                        nc.tensor.matmul(ps[:, :], w2[:, k, n * 128:(n + 1) * 128],
                                         h[k][:, :], start=(k == 0), stop=(k == NK2 - 1))
                    nc.scalar.activation(ml[n][:, :], ps[:, :], AF.Identity,
                                         bias=b2t[:, n:n + 1])

                # -- L3: HD = relu(mu@W3 + crep@W3c + b3) --
                mu_b = mlp.tile([128, RB], MMDT, tag="mub")
                nc.vector.tensor_copy(mu_b[:, :], ml[0][:, :])
                hd = [hdp.tile([128, RB], MMDT, tag="hd", name=f"hd{_k}") for _k in range(NN1)]
                for n in range(NN1):
                    ps = psmm.tile([128, RB], F32, tag="mm")
                    nc.tensor.matmul(ps[:, :], w3[:, n * 128:(n + 1) * 128],
                                     mu_b[:, :], start=True, stop=False)
                    nc.tensor.matmul(ps[:, :], w3c[:, n * 128:(n + 1) * 128],
                                     crep[:, rbs], start=False, stop=True)
                    nc.scalar.activation(hd[n][:, :], ps[:, :], AF.Relu,
                                         bias=b3t[:, n:n + 1])

                # -- L4: P = sigmoid(HD@W4 + b4) --
                p = [pp.tile([128, RB], MMDT, tag="p", name=f"p{_k}") for _k in range(NN4)]
                for n in range(NN4):
                    ps = psmm.tile([128, RB], F32, tag="mm")
                    for k in range(NK2):
                        nc.tensor.matmul(ps[:, :], w4[:, k, n * 128:(n + 1) * 128],
                                         hd[k][:, :], start=(k == 0), stop=(k == NK2 - 1))
                    nc.scalar.activation(p[n][:, :], ps[:, :], AF.Sigmoid,
                                         bias=b4t[:, n:n + 1])

                # -- Phase C: transpose back + DMA out (ML first) --
                for n2 in range(NN2):
                    for j in range(OH):
                        ps = pstp.tile([128, 128], F32, tag="tp")
                        nc.tensor.transpose(
                            ps[:, :], ml[n2][:, j * 128:(j + 1) * 128],
                            ident[:, :])
                        nc.vector.tensor_copy(
                            mlrm[:, n2 * 128:(n2 + 1) * 128, i * OH + j], ps[:, :])

                if i == OH - 1:
                    nc.sync.dma_start(ml_d[:, :, :, :],
                                      mlrm.rearrange("b n (i j) -> b n i j", i=OH))
                prm = prmp.tile([128, CH, K, OH, K], F32, tag="prm")
                for kc in range(NN4):
                    ch, rh = kc // 2, kc % 2
                    for j in range(OH):
                        ps = pstp.tile([128, 128], MMDT, tag="tp")
                        nc.tensor.transpose(
                            ps[:, :], p[kc][:, j * 128:(j + 1) * 128],
                            identm[:, :])
                        nc.vector.tensor_copy(
                            prm[:, ch, rh * 8:(rh + 1) * 8, j, :], ps[:, :])
                for ch in range(CH):
                    nc.sync.dma_start(recon_d[:, ch, K * i:K * (i + 1), :],
                                      prm[:, ch, :, :, :])


    nc.compile()
    return nc


def _prep_maps(x, c, enc_w1, enc_b1, enc_w2, enc_b2, dec_w1, dec_b1, dec_w2, dec_b2):
    f = np.ascontiguousarray
    mmnp = ml_dtypes.bfloat16 if MMDT == BF16 else np.float32
    shared = {
        "w1": f(enc_w1[:PD], mmnp),
        "w1c": f(enc_w1[PD:], mmnp),
        "w2": f(enc_w2, mmnp),
        "w3": f(dec_w1[:LAT], mmnp),
        "w3c": f(dec_w1[LAT:], mmnp),
        "w4": f(dec_w2, mmnp),
        "b1t": f(enc_b1.reshape(-1, 128).T, np.float32),
        "b2t": f(enc_b2.reshape(-1, 128).T, np.float32),
        "b3t": f(dec_b1.reshape(-1, 128).T, np.float32),
        "b4t": f(dec_b2.reshape(-1, 128).T, np.float32),
    }
    in_maps = []
    for ci in range(N_CORES):
        xs = f(x[ci * B:(ci + 1) * B], np.float32)
        cs = c[ci * B:(ci + 1) * B]  # [B, NCLS]
        crep = f(np.tile(cs.T.astype(np.float32), (1, L)).astype(mmnp))  # [NCLS, L*B]
        in_maps.append({"x": xs, "crep": crep, **shared})
    return in_maps


def kernel(x, c, enc_w1, enc_b1, enc_w2, enc_b2, dec_w1, dec_b1, dec_w2, dec_b2,
           _trace=False):
    global _cached
    if _cached is None:
        _cached = _build()
    nc = _cached
    in_maps = _prep_maps(np.asarray(x), np.asarray(c), np.asarray(enc_w1),
                         np.asarray(enc_b1), np.asarray(enc_w2), np.asarray(enc_b2),
                         np.asarray(dec_w1), np.asarray(dec_b1), np.asarray(dec_w2),
                         np.asarray(dec_b2))
    res = bass_utils.run_bass_kernel_spmd(nc, in_maps, core_ids=list(range(N_CORES)),
                                          trace=_trace)
    recon = np.concatenate([r["recon"] for r in res.results], axis=0)
    ml = np.concatenate([r["ml"] for r in res.results], axis=0)
    if _trace:
        kernel.last_results = res
    return recon, ml


# revision 13
# speedup vs baseline: 1.1231x; 1.1231x over previous
"""ConvVAE2d (patchify -> CVAE MLP encode/decode -> fold) on 8 TRN2 NeuronCores.

Data-parallel over batch: 1024 samples -> 8 cores x 128 samples.

Per-core dataflow (128 samples, 16 patches each -> 2048 MLP rows):
  All activations are kept transposed [feature, row] in SBUF, with rows
  ordered (ij, b) = patch-position-major. Weights act as the stationary
  matmul operand (lhsT), so every layer consumes and produces the same
  layout and no transposes are needed between layers.

  Phase A (per row-block i): DMA x i-slice -> PE-transpose 128x128 blocks
           into XT [768, 512] (patchify == transpose).
  Phase B: L1 relu(XT@W1 + crep@W1c + b1) -> H [1024, 512]
           L2 H@W2 + b2 -> ML [256, 512]   (mu = first 128 rows)
           L3 relu(mu@W3 + crep@W3c + b3) -> HD [1024, 512]
           L4 sigmoid(HD@W4 + b4) -> P [768, 512]
  Phase C: PE-transpose P / ML back to row-major staging, DMA out.

  Matmuls run as float32r (full PE speed, ~1e-4 rel err). Biases and
  activation functions are fused into the PSUM->SBUF eviction on ScalarE.
"""
import sys

try:
    import concourse.bass as bass
except ImportError:
    sys.path.insert(0, "/opt/trn_rl_repo")
    import concourse.bass as bass

import ml_dtypes
import numpy as np
import concourse.bacc as bacc
import concourse.mybir as mybir
import concourse.tile as tile
from concourse import bass_utils
from concourse.masks import make_identity

# Problem config (hardcoded; must match the reference)
BS, CH, IMG, K, NCLS, HID, LAT = 1024, 3, 64, 16, 10, 1024, 128
OH = IMG // K          # 4 patches per side
L = OH * OH            # 16 patches per image
PD = CH * K * K        # 768 flattened patch dim
N_CORES = 8
B = BS // N_CORES      # 128 samples per core
ROWS = B * L           # 2048 MLP rows per core
RB = 512               # rows per row-block (= 4 patch positions x 128 samples)

F32 = mybir.dt.float32
F32R = mybir.dt.float32r
BF16 = mybir.dt.bfloat16
MMDT = BF16  # matmul operand dtype (1 cyc/elem on PE; f32r runs at 2 cyc/elem)
AF = mybir.ActivationFunctionType

_cached = None


def _build():
    nc = bacc.Bacc("TRN2", target_bir_lowering=False, debug=False,
                   num_devices=N_CORES)

    def din(name, shape, dt=MMDT):
        return nc.dram_tensor(name, shape, dt, kind="ExternalInput").ap()

    x_d = din("x", [B, CH, IMG, IMG], F32)
    crep_d = din("crep", [NCLS, ROWS])
    w1_d = din("w1", [PD, HID])
    w1c_d = din("w1c", [NCLS, HID])
    w2_d = din("w2", [HID, 2 * LAT])
    w3_d = din("w3", [LAT, HID])
    w3c_d = din("w3c", [NCLS, HID])
    w4_d = din("w4", [HID, PD])
    b1_d = din("b1t", [128, HID // 128], F32)
    b2_d = din("b2t", [128, 2 * LAT // 128], F32)
    b3_d = din("b3t", [128, HID // 128], F32)
    b4_d = din("b4t", [128, PD // 128], F32)
    recon_d = nc.dram_tensor("recon", [B, CH, IMG, IMG], F32,
                             kind="ExternalOutput").ap()
    ml_d = nc.dram_tensor("ml", [B, 2 * LAT, OH, OH], F32,
                          kind="ExternalOutput").ap()

    NK1 = PD // 128    # 6 k-chunks for layer 1
    NN1 = HID // 128   # 8 n-chunks for layers 1/3
    NK2 = HID // 128   # 8
    NN2 = 2 * LAT // 128  # 2
    NN4 = PD // 128    # 6

    with tile.TileContext(nc) as tc:
        with (
            tc.tile_pool(name="const", bufs=1) as cp,
            tc.tile_pool(name="xnat", bufs=2) as xnp,
            tc.tile_pool(name="ximg", bufs=2) as xip,
            tc.tile_pool(name="xt", bufs=2 * NK1 + 2) as xtp,
            tc.tile_pool(name="h", bufs=NN1 + 2) as hp,
            tc.tile_pool(name="ml", bufs=2 * NN2 + 1) as mlp,
            tc.tile_pool(name="hd", bufs=NN1 + 2) as hdp,
            tc.tile_pool(name="p", bufs=4) as pp,
            tc.tile_pool(name="prm", bufs=2) as prmp,
            tc.tile_pool(name="psmm", bufs=4, space="PSUM") as psmm,
            tc.tile_pool(name="pstp", bufs=4, space="PSUM") as pstp,
        ):
            # ---- constants ----
            # DMA issue order = first-use order: x row-block 0 first so the
            # patchify transposes start immediately, then per-layer weights
            # just-in-time (w1 before w2 before w3/w4).
            ident = cp.tile([128, 128], F32)
            make_identity(nc, ident[:, :])
            identm = cp.tile([128, 128], MMDT)
            make_identity(nc, identm[:, :])
            xnat0 = xnp.tile([128, CH, K, IMG], F32, tag="xnat", name="xnat0")
            w1 = cp.tile([128, NK1, HID], MMDT)
            w1v = w1_d.rearrange("(k p) n -> p k n", p=128)
            for ch in range(CH):
                nc.sync.dma_start(xnat0[:, ch, :, :], x_d[:, ch, 0:K, :])
                nc.sync.dma_start(w1[:, 2 * ch, :], w1v[:, 2 * ch, :])
                nc.sync.dma_start(w1[:, 2 * ch + 1, :], w1v[:, 2 * ch + 1, :])
            w1c = cp.tile([NCLS, HID], MMDT)
            nc.sync.dma_start(w1c[:, :], w1c_d)
            crep = cp.tile([NCLS, ROWS], MMDT)
            nc.sync.dma_start(crep[:, :], crep_d)
            b1t = cp.tile([128, NN1], F32)
            nc.sync.dma_start(b1t[:, :], b1_d)
            w2 = cp.tile([128, NK2, 2 * LAT], MMDT)
            nc.sync.dma_start(w2[:, :, :], w2_d.rearrange("(k p) n -> p k n", p=128))
            b2t = cp.tile([128, NN2], F32)
            nc.sync.dma_start(b2t[:, :], b2_d)
            w3 = cp.tile([128, HID], MMDT)
            nc.sync.dma_start(w3[:, :], w3_d)
            w3c = cp.tile([NCLS, HID], MMDT)
            nc.sync.dma_start(w3c[:, :], w3c_d)
            b3t = cp.tile([128, NN1], F32)
            nc.sync.dma_start(b3t[:, :], b3_d)
            w4 = cp.tile([128, NK2, PD], MMDT)
            nc.sync.dma_start(w4[:, :, :], w4_d.rearrange("(k p) n -> p k n", p=128))
            b4t = cp.tile([128, NN4], F32)
            nc.sync.dma_start(b4t[:, :], b4_d)
            # mu_logvar row-major staging, viewed [b, n, ij]
            mlrm = cp.tile([128, 2 * LAT, L], F32)

            # Patchify transposes are latency-bound and do NOT count as
            # PE-busy for the HAM clock gate: a >3.4us matmul-free transpose
            # burst re-throttles the PE to 1.2 GHz. So transposes are
            # interleaved into the matmul n-loops in small groups:
            #   - rb i+1 input transposes ride inside rb i's L1 loop
            #   - ML output transposes ride inside the L3 loop
            #   - P output transposes follow their own L4 n-chunk eviction
            def repack(i, xnat):
                """DVE-repack x i-slice into transpose-ready (ch,j,rh,rc)."""
                ximg = xip.tile([128, CH, OH, 2, 128], MMDT, tag="ximg",
                                name=f"ximg{i}")
                for ch in range(CH):
                    for rh in range(2):
                        nc.vector.tensor_copy(
                            ximg[:, ch, :, rh, :].rearrange(
                                "b j (r c) -> b j r c", c=K),
                            xnat[:, ch, 8 * rh:8 * rh + 8, :].rearrange(
                                "b r (j c) -> b j r c", c=K))
                return ximg

            def in_tp(ximg, xt, pairs):
                for kc, j in pairs:
                    ch, rh = kc // 2, kc % 2
                    ps = pstp.tile([128, 128], MMDT, tag="tp")
                    nc.tensor.transpose(ps[:, :], ximg[:, ch, j, rh, :],
                                        identm[:, :])
                    nc.vector.tensor_copy(xt[kc][:, j * 128:(j + 1) * 128],
                                          ps[:, :])

            def new_xt():
                return [xtp.tile([128, RB], MMDT, tag="xt", name=f"xt{_k}")
                        for _k in range(NK1)]

            ximg_c = repack(0, xnat0)
            xt_c = new_xt()
            in_tp(ximg_c, xt_c, [(kc, j) for kc in range(NK1) for j in range(OH)])

            for i in range(OH):  # row-block = one patch-row index i
                if i + 1 < OH:
                    xnat = xnp.tile([128, CH, K, IMG], F32, tag="xnat")
                    nc.sync.dma_start(xnat[:, :, :, :],
                                      x_d[:, :, K * (i + 1):K * (i + 2), :])
                    ximg_n = repack(i + 1, xnat)
                    xt_n = new_xt()
                    tp_feed = [(kc, j) for kc in range(NK1) for j in range(OH)]
                else:
                    ximg_n, xt_n, tp_feed = None, None, []
                xt = xt_c
                rbs = slice(i * RB, (i + 1) * RB)

                # -- L1: H = relu(XT@W1 + crep@W1c + b1) --
                h = [hp.tile([128, RB], MMDT, tag="h", name=f"h{_k}")
                     for _k in range(NN1)]
                for n in range(NN1):
                    ps = psmm.tile([128, RB], F32, tag="mm")
                    for k in range(NK1):
                        nc.tensor.matmul(ps[:, :], w1[:, k, n * 128:(n + 1) * 128],
                                         xt[k][:, :], start=(k == 0), stop=False)
                    nc.tensor.matmul(ps[:, :], w1c[:, n * 128:(n + 1) * 128],
                                     crep[:, rbs], start=False, stop=True)
                    nc.scalar.activation(h[n][:, :], ps[:, :], AF.Relu,
                                         bias=b1t[:, n:n + 1])
                    if tp_feed:
                        in_tp(ximg_n, xt_n, tp_feed[3 * n:3 * n + 3])

                # -- L2: ML = H@W2 + b2 --
                ml = [mlp.tile([128, RB], F32, tag="ml", name=f"ml{_k}")
                      for _k in range(NN2)]
                for n in range(NN2):
                    ps = psmm.tile([128, RB], F32, tag="mm")
                    for k in range(NK2):
                        nc.tensor.matmul(ps[:, :], w2[:, k, n * 128:(n + 1) * 128],
                                         h[k][:, :], start=(k == 0),
                                         stop=(k == NK2 - 1))
                    nc.scalar.activation(ml[n][:, :], ps[:, :], AF.Identity,
                                         bias=b2t[:, n:n + 1])

                # -- L3: HD = relu(mu@W3 + crep@W3c + b3) --
                # (one ML output transpose rides after each n-chunk)
                mu_b = mlp.tile([128, RB], MMDT, tag="mub")
                nc.vector.tensor_copy(mu_b[:, :], ml[0][:, :])
                hd = [hdp.tile([128, RB], MMDT, tag="hd", name=f"hd{_k}")
                      for _k in range(NN1)]
                for n in range(NN1):
                    ps = psmm.tile([128, RB], F32, tag="mm")
                    nc.tensor.matmul(ps[:, :], w3[:, n * 128:(n + 1) * 128],
                                     mu_b[:, :], start=True, stop=False)
                    nc.tensor.matmul(ps[:, :], w3c[:, n * 128:(n + 1) * 128],
                                     crep[:, rbs], start=False, stop=True)
                    nc.scalar.activation(hd[n][:, :], ps[:, :], AF.Relu,
                                         bias=b3t[:, n:n + 1])
                    n2, j = n // OH, n % OH
                    psT = pstp.tile([128, 128], F32, tag="tp")
                    nc.tensor.transpose(psT[:, :],
                                        ml[n2][:, j * 128:(j + 1) * 128],
                                        ident[:, :])
                    nc.vector.tensor_copy(
                        mlrm[:, n2 * 128:(n2 + 1) * 128, i * OH + j], psT[:, :])

                if i == OH - 1:
                    nc.sync.dma_start(ml_d[:, :, :, :],
                                      mlrm.rearrange("b n (i j) -> b n i j", i=OH))

                # -- L4: P = sigmoid(HD@W4 + b4), transpose-out per n-chunk --
                prm = prmp.tile([128, CH, K, OH, K], F32, tag="prm")
                for n in range(NN4):
                    ps = psmm.tile([128, RB], F32, tag="mm")
                    for k in range(NK2):
                        nc.tensor.matmul(ps[:, :], w4[:, k, n * 128:(n + 1) * 128],
                                         hd[k][:, :], start=(k == 0),
                                         stop=(k == NK2 - 1))
                    pn = pp.tile([128, RB], MMDT, tag="p")
                    nc.scalar.activation(pn[:, :], ps[:, :], AF.Sigmoid,
                                         bias=b4t[:, n:n + 1])
                    ch, rh = n // 2, n % 2
                    for j in range(OH):
                        psT = pstp.tile([128, 128], MMDT, tag="tp")
                        nc.tensor.transpose(psT[:, :],
                                            pn[:, j * 128:(j + 1) * 128],
                                            identm[:, :])
                        nc.vector.tensor_copy(
                            prm[:, ch, rh * 8:(rh + 1) * 8, j, :], psT[:, :])
                nc.sync.dma_start(recon_d[:, :, K * i:K * (i + 1), :],
                                  prm[:, :, :, :, :])

                ximg_c, xt_c = ximg_n, xt_n

    nc.compile()
    return nc


def _prep_maps(x, c, enc_w1, enc_b1, enc_w2, enc_b2, dec_w1, dec_b1, dec_w2, dec_b2):
    f = np.ascontiguousarray
    mmnp = ml_dtypes.bfloat16 if MMDT == BF16 else np.float32
    shared = {
        "w1": f(enc_w1[:PD], mmnp),
        "w1c": f(enc_w1[PD:], mmnp),
        "w2": f(enc_w2, mmnp),
        "w3": f(dec_w1[:LAT], mmnp),
        "w3c": f(dec_w1[LAT:], mmnp),
        "w4": f(dec_w2, mmnp),
        "b1t": f(enc_b1.reshape(-1, 128).T, np.float32),
        "b2t": f(enc_b2.reshape(-1, 128).T, np.float32),
        "b3t": f(dec_b1.reshape(-1, 128).T, np.float32),
        "b4t": f(dec_b2.reshape(-1, 128).T, np.float32),
    }
    in_maps = []
    for ci in range(N_CORES):
        xs = f(x[ci * B:(ci + 1) * B], np.float32)
        cs = c[ci * B:(ci + 1) * B]  # [B, NCLS]
        crep = f(np.tile(cs.T.astype(np.float32), (1, L)).astype(mmnp))  # [NCLS, L*B]
        in_maps.append({"x": xs, "crep": crep, **shared})
    return in_maps


def kernel(x, c, enc_w1, enc_b1, enc_w2, enc_b2, dec_w1, dec_b1, dec_w2, dec_b2,
           _trace=False):
    global _cached
    if _cached is None:
        _cached = _build()
    nc = _cached
    in_maps = _prep_maps(np.asarray(x), np.asarray(c), np.asarray(enc_w1),
                         np.asarray(enc_b1), np.asarray(enc_w2), np.asarray(enc_b2),
                         np.asarray(dec_w1), np.asarray(dec_b1), np.asarray(dec_w2),
                         np.asarray(dec_b2))
    res = bass_utils.run_bass_kernel_spmd(nc, in_maps, core_ids=list(range(N_CORES)),
                                          trace=_trace)
    recon = np.concatenate([r["recon"] for r in res.results], axis=0)
    ml = np.concatenate([r["ml"] for r in res.results], axis=0)
    if _trace:
        kernel.last_results = res
    return recon, ml


# revision 14
# speedup vs baseline: 1.1456x; 1.0200x over previous
"""ConvVAE2d (patchify -> CVAE MLP encode/decode -> fold) on 8 TRN2 NeuronCores.

Data-parallel over batch: 1024 samples -> 8 cores x 128 samples.

Per-core dataflow (128 samples, 16 patches each -> 2048 MLP rows):
  All activations are kept transposed [feature, row] in SBUF, with rows
  ordered (ij, b) = patch-position-major. Weights act as the stationary
  matmul operand (lhsT), so every layer consumes and produces the same
  layout and no transposes are needed between layers.

  Phase A (per row-block i): DMA x i-slice -> PE-transpose 128x128 blocks
           into XT [768, 512] (patchify == transpose).
  Phase B: L1 relu(XT@W1 + crep@W1c + b1) -> H [1024, 512]
           L2 H@W2 + b2 -> ML [256, 512]   (mu = first 128 rows)
           L3 relu(mu@W3 + crep@W3c + b3) -> HD [1024, 512]
           L4 sigmoid(HD@W4 + b4) -> P [768, 512]
  Phase C: PE-transpose P / ML back to row-major staging, DMA out.

  Matmuls run as float32r (full PE speed, ~1e-4 rel err). Biases and
  activation functions are fused into the PSUM->SBUF eviction on ScalarE.
"""
import sys

try:
    import concourse.bass as bass
except ImportError:
    sys.path.insert(0, "/opt/trn_rl_repo")
    import concourse.bass as bass

import ml_dtypes
import numpy as np
import concourse.bacc as bacc
import concourse.mybir as mybir
import concourse.tile as tile
from concourse import bass_utils
from concourse.masks import make_identity

# Problem config (hardcoded; must match the reference)
BS, CH, IMG, K, NCLS, HID, LAT = 1024, 3, 64, 16, 10, 1024, 128
OH = IMG // K          # 4 patches per side
L = OH * OH            # 16 patches per image
PD = CH * K * K        # 768 flattened patch dim
N_CORES = 8
B = BS // N_CORES      # 128 samples per core
ROWS = B * L           # 2048 MLP rows per core
RB = 512               # rows per row-block (= 4 patch positions x 128 samples)

F32 = mybir.dt.float32
F32R = mybir.dt.float32r
BF16 = mybir.dt.bfloat16
MMDT = BF16  # matmul operand dtype (1 cyc/elem on PE; f32r runs at 2 cyc/elem)
AF = mybir.ActivationFunctionType

_cached = None


def _build():
    nc = bacc.Bacc("TRN2", target_bir_lowering=False, debug=False,
                   num_devices=N_CORES)

    def din(name, shape, dt=MMDT):
        return nc.dram_tensor(name, shape, dt, kind="ExternalInput").ap()

    x_d = din("x", [B, CH, IMG, IMG], F32)
    crep_d = din("crep", [NCLS, ROWS])
    w1_d = din("w1", [PD, HID])
    w1c_d = din("w1c", [NCLS, HID])
    w2_d = din("w2", [HID, 2 * LAT])
    w3_d = din("w3", [LAT, HID])
    w3c_d = din("w3c", [NCLS, HID])
    w4_d = din("w4", [HID, PD])
    b1_d = din("b1t", [128, HID // 128], F32)
    b2_d = din("b2t", [128, 2 * LAT // 128], F32)
    b3_d = din("b3t", [128, HID // 128], F32)
    b4_d = din("b4t", [128, PD // 128], F32)
    recon_d = nc.dram_tensor("recon", [B, CH, IMG, IMG], F32,
                             kind="ExternalOutput").ap()
    ml_d = nc.dram_tensor("ml", [B, 2 * LAT, OH, OH], F32,
                          kind="ExternalOutput").ap()

    NK1 = PD // 128    # 6 k-chunks for layer 1
    NN1 = HID // 128   # 8 n-chunks for layers 1/3
    NK2 = HID // 128   # 8
    NN2 = 2 * LAT // 128  # 2
    NN4 = PD // 128    # 6

    with tile.TileContext(nc) as tc:
        with (
            tc.tile_pool(name="const", bufs=1) as cp,
            tc.tile_pool(name="xnat", bufs=2) as xnp,
            tc.tile_pool(name="ximg", bufs=2) as xip,
            tc.tile_pool(name="xt", bufs=2 * NK1 + 2) as xtp,
            tc.tile_pool(name="h", bufs=NN1 + 2) as hp,
            tc.tile_pool(name="ml", bufs=2 * NN2 + 1) as mlp,
            tc.tile_pool(name="hd", bufs=NN1 + 2) as hdp,
            tc.tile_pool(name="p", bufs=4) as pp,
            tc.tile_pool(name="prm", bufs=2) as prmp,
            tc.tile_pool(name="psmm", bufs=4, space="PSUM") as psmm,
            tc.tile_pool(name="pstp", bufs=4, space="PSUM") as pstp,
        ):
            # ---- constants ----
            # DMA issue order = first-use order: x row-block 0 first so the
            # patchify transposes start immediately, then per-layer weights
            # just-in-time (w1 before w2 before w3/w4).
            ident = cp.tile([128, 128], F32)
            make_identity(nc, ident[:, :])
            identm = cp.tile([128, 128], MMDT)
            make_identity(nc, identm[:, :])
            xnat0 = xnp.tile([128, CH, K, IMG], F32, tag="xnat", name="xnat0")
            w1 = cp.tile([128, NK1, HID], MMDT)
            w1v = w1_d.rearrange("(k p) n -> p k n", p=128)
            nc.sync.dma_start(xnat0[:, 0, :, :], x_d[:, 0, 0:K, :])
            w1c = cp.tile([NCLS, HID], MMDT)
            nc.sync.dma_start(w1c[:, :], w1c_d)
            crep = cp.tile([NCLS, ROWS], MMDT)
            nc.sync.dma_start(crep[:, :], crep_d)
            b1t = cp.tile([128, NN1], F32)
            nc.sync.dma_start(b1t[:, :], b1_d)
            for ch in range(CH):
                if ch:
                    nc.sync.dma_start(xnat0[:, ch, :, :], x_d[:, ch, 0:K, :])
                nc.sync.dma_start(w1[:, 2 * ch, :], w1v[:, 2 * ch, :])
                nc.sync.dma_start(w1[:, 2 * ch + 1, :], w1v[:, 2 * ch + 1, :])
            w2 = cp.tile([128, NK2, 2 * LAT], MMDT)
            nc.sync.dma_start(w2[:, :, :], w2_d.rearrange("(k p) n -> p k n", p=128))
            b2t = cp.tile([128, NN2], F32)
            nc.sync.dma_start(b2t[:, :], b2_d)
            w3 = cp.tile([128, HID], MMDT)
            nc.sync.dma_start(w3[:, :], w3_d)
            w3c = cp.tile([NCLS, HID], MMDT)
            nc.sync.dma_start(w3c[:, :], w3c_d)
            b3t = cp.tile([128, NN1], F32)
            nc.sync.dma_start(b3t[:, :], b3_d)
            w4 = cp.tile([128, NK2, PD], MMDT)
            nc.sync.dma_start(w4[:, :, :], w4_d.rearrange("(k p) n -> p k n", p=128))
            b4t = cp.tile([128, NN4], F32)
            nc.sync.dma_start(b4t[:, :], b4_d)
            # mu_logvar row-major staging, viewed [b, n, ij]
            mlrm = cp.tile([128, 2 * LAT, L], F32)

            # Patchify transposes are latency-bound and do NOT count as
            # PE-busy for the HAM clock gate: a >3.4us matmul-free transpose
            # burst re-throttles the PE to 1.2 GHz. So transposes are
            # interleaved into the matmul n-loops in small groups:
            #   - rb i+1 input transposes ride inside rb i's L1 loop
            #   - ML output transposes ride inside the L3 loop
            #   - P output transposes follow their own L4 n-chunk eviction
            def repack(i, xnat):
                """DVE-repack x i-slice into transpose-ready (ch,j,rh,rc)."""
                ximg = xip.tile([128, CH, OH, 2, 128], MMDT, tag="ximg",
                                name=f"ximg{i}")
                for ch in range(CH):
                    for rh in range(2):
                        nc.vector.tensor_copy(
                            ximg[:, ch, :, rh, :].rearrange(
                                "b j (r c) -> b j r c", c=K),
                            xnat[:, ch, 8 * rh:8 * rh + 8, :].rearrange(
                                "b r (j c) -> b j r c", c=K))
                return ximg

            def in_tp(ximg, xt, pairs):
                for kc, j in pairs:
                    ch, rh = kc // 2, kc % 2
                    ps = pstp.tile([128, 128], MMDT, tag="tp")
                    nc.tensor.transpose(ps[:, :], ximg[:, ch, j, rh, :],
                                        identm[:, :])
                    nc.vector.tensor_copy(xt[kc][:, j * 128:(j + 1) * 128],
                                          ps[:, :])

            def new_xt():
                return [xtp.tile([128, RB], MMDT, tag="xt", name=f"xt{_k}")
                        for _k in range(NK1)]

            ximg_c = repack(0, xnat0)
            xt_c = new_xt()
            in_tp(ximg_c, xt_c, [(kc, j) for kc in range(NK1) for j in range(OH)])

            for i in range(OH):  # row-block = one patch-row index i
                if i + 1 < OH:
                    xnat = xnp.tile([128, CH, K, IMG], F32, tag="xnat")
                    nc.sync.dma_start(xnat[:, :, :, :],
                                      x_d[:, :, K * (i + 1):K * (i + 2), :])
                    ximg_n = repack(i + 1, xnat)
                    xt_n = new_xt()
                    tp_feed = [(kc, j) for kc in range(NK1) for j in range(OH)]
                else:
                    ximg_n, xt_n, tp_feed = None, None, []
                xt = xt_c
                rbs = slice(i * RB, (i + 1) * RB)

                # -- L1: H = relu(XT@W1 + crep@W1c + b1) --
                h = [hp.tile([128, RB], MMDT, tag="h", name=f"h{_k}")
                     for _k in range(NN1)]
                for n in range(NN1):
                    ps = psmm.tile([128, RB], F32, tag="mm")
                    for k in range(NK1):
                        nc.tensor.matmul(ps[:, :], w1[:, k, n * 128:(n + 1) * 128],
                                         xt[k][:, :], start=(k == 0), stop=False)
                    nc.tensor.matmul(ps[:, :], w1c[:, n * 128:(n + 1) * 128],
                                     crep[:, rbs], start=False, stop=True)
                    nc.scalar.activation(h[n][:, :], ps[:, :], AF.Relu,
                                         bias=b1t[:, n:n + 1])
                    if tp_feed:
                        in_tp(ximg_n, xt_n, tp_feed[3 * n:3 * n + 3])

                # -- L2: ML = H@W2 + b2 --
                ml = [mlp.tile([128, RB], F32, tag="ml", name=f"ml{_k}")
                      for _k in range(NN2)]
                for n in range(NN2):
                    ps = psmm.tile([128, RB], F32, tag="mm")
                    for k in range(NK2):
                        nc.tensor.matmul(ps[:, :], w2[:, k, n * 128:(n + 1) * 128],
                                         h[k][:, :], start=(k == 0),
                                         stop=(k == NK2 - 1))
                    nc.scalar.activation(ml[n][:, :], ps[:, :], AF.Identity,
                                         bias=b2t[:, n:n + 1])

                # -- L3: HD = relu(mu@W3 + crep@W3c + b3) --
                # (one ML output transpose rides after each n-chunk)
                mu_b = mlp.tile([128, RB], MMDT, tag="mub")
                nc.vector.tensor_copy(mu_b[:, :], ml[0][:, :])
                hd = [hdp.tile([128, RB], MMDT, tag="hd", name=f"hd{_k}")
                      for _k in range(NN1)]
                for n in range(NN1):
                    ps = psmm.tile([128, RB], F32, tag="mm")
                    nc.tensor.matmul(ps[:, :], w3[:, n * 128:(n + 1) * 128],
                                     mu_b[:, :], start=True, stop=False)
                    nc.tensor.matmul(ps[:, :], w3c[:, n * 128:(n + 1) * 128],
                                     crep[:, rbs], start=False, stop=True)
                    nc.scalar.activation(hd[n][:, :], ps[:, :], AF.Relu,
                                         bias=b3t[:, n:n + 1])
                    n2, j = n // OH, n % OH
                    psT = pstp.tile([128, 128], F32, tag="tp")
                    nc.tensor.transpose(psT[:, :],
                                        ml[n2][:, j * 128:(j + 1) * 128],
                                        ident[:, :])
                    nc.vector.tensor_copy(
                        mlrm[:, n2 * 128:(n2 + 1) * 128, i * OH + j], psT[:, :])

                if i == OH - 1:
                    nc.sync.dma_start(ml_d[:, :, :, :],
                                      mlrm.rearrange("b n (i j) -> b n i j", i=OH))

                # -- L4: P = sigmoid(HD@W4 + b4), transpose-out per n-chunk --
                prm = prmp.tile([128, CH, K, OH, K], F32, tag="prm")
                for n in range(NN4):
                    ps = psmm.tile([128, RB], F32, tag="mm")
                    for k in range(NK2):
                        nc.tensor.matmul(ps[:, :], w4[:, k, n * 128:(n + 1) * 128],
                                         hd[k][:, :], start=(k == 0),
                                         stop=(k == NK2 - 1))
                    pn = pp.tile([128, RB], MMDT, tag="p")
                    nc.scalar.activation(pn[:, :], ps[:, :], AF.Sigmoid,
                                         bias=b4t[:, n:n + 1])
                    ch, rh = n // 2, n % 2
                    for j in range(OH):
                        psT = pstp.tile([128, 128], MMDT, tag="tp")
                        nc.tensor.transpose(psT[:, :],
                                            pn[:, j * 128:(j + 1) * 128],
                                            identm[:, :])
                        nc.vector.tensor_copy(
                            prm[:, ch, rh * 8:(rh + 1) * 8, j, :], psT[:, :])
                if i == OH - 1:
                    for ch in range(CH):
                        nc.sync.dma_start(recon_d[:, ch, K * i:K * (i + 1), :],
                                          prm[:, ch, :, :, :])
                else:
                    nc.sync.dma_start(recon_d[:, :, K * i:K * (i + 1), :],
                                      prm[:, :, :, :, :])

                ximg_c, xt_c = ximg_n, xt_n

    nc.compile()
    return nc


def _prep_maps(x, c, enc_w1, enc_b1, enc_w2, enc_b2, dec_w1, dec_b1, dec_w2, dec_b2):
    f = np.ascontiguousarray
    mmnp = ml_dtypes.bfloat16 if MMDT == BF16 else np.float32
    shared = {
        "w1": f(enc_w1[:PD], mmnp),
        "w1c": f(enc_w1[PD:], mmnp),
        "w2": f(enc_w2, mmnp),
        "w3": f(dec_w1[:LAT], mmnp),
        "w3c": f(dec_w1[LAT:], mmnp),
        "w4": f(dec_w2, mmnp),
        "b1t": f(enc_b1.reshape(-1, 128).T, np.float32),
        "b2t": f(enc_b2.reshape(-1, 128).T, np.float32),
        "b3t": f(dec_b1.reshape(-1, 128).T, np.float32),
        "b4t": f(dec_b2.reshape(-1, 128).T, np.float32),
    }
    in_maps = []
    for ci in range(N_CORES):
        xs = f(x[ci * B:(ci + 1) * B], np.float32)
        cs = c[ci * B:(ci + 1) * B]  # [B, NCLS]
        crep = f(np.tile(cs.T.astype(np.float32), (1, L)).astype(mmnp))  # [NCLS, L*B]
        in_maps.append({"x": xs, "crep": crep, **shared})
    return in_maps


def kernel(x, c, enc_w1, enc_b1, enc_w2, enc_b2, dec_w1, dec_b1, dec_w2, dec_b2,
           _trace=False):
    global _cached
    if _cached is None:
        _cached = _build()
    nc = _cached
    in_maps = _prep_maps(np.asarray(x), np.asarray(c), np.asarray(enc_w1),
                         np.asarray(enc_b1), np.asarray(enc_w2), np.asarray(enc_b2),
                         np.asarray(dec_w1), np.asarray(dec_b1), np.asarray(dec_w2),
                         np.asarray(dec_b2))
    res = bass_utils.run_bass_kernel_spmd(nc, in_maps, core_ids=list(range(N_CORES)),
                                          trace=_trace)
    recon = np.concatenate([r["recon"] for r in res.results], axis=0)
    ml = np.concatenate([r["ml"] for r in res.results], axis=0)
    if _trace:
        kernel.last_results = res
    return recon, ml
